# revision 55
# baseline (speedup 1.0000x reference)
"""GAT-with-gate kernel for Trainium2 (8 NeuronCores).

Row-shards the 8192 nodes across 8 cores (1024 rows each). Each core:
  h^T = W x^T (+b), h_aug = [x W^T + b | 1]          (small matmuls, f32r)
  g^T = A_sym^T h_loc^T          with A_sym = A + A^T (so e+e^T needs no transpose)
  for each 128-wide j-block:  e^T[j,i] = h^T[:,j]^T g^T   (PE, f32r)
      p^T = adj^T(j,i) * exp(e^T - 20)   (mask via GPSIMD local_scatter, exp on ACT)
  h'pre[i,:]|Z[i] = sum_j p^T[j,i] * [h(j,:) | 1]  (PE, bf16 p/h_aug, ones-column)
  h' = relu(h'pre / Z);  coeff = sigmoid([x,h'] @ gate_w^T + b)
  out = coeff*x + (1-coeff)*h'
"""
import os
import sys

import numpy as np

for _p in ("/opt/trn_rl_repo", "/root/.axon_site/_ro/trn_rl_repo"):
    if os.path.isdir(_p) and _p not in sys.path:
        sys.path.append(_p)

import concourse.bass as bass  # noqa: E402
import concourse.mybir as mybir  # noqa: E402
import concourse.tile as tile  # noqa: E402
from concourse import bacc, library_config  # noqa: E402
from concourse.bass_utils import run_bass_kernel_spmd  # noqa: E402

N = 8192
D = 256
M = 8          # cores
NL = N // M    # 1024 local rows per core
P = 128
JBLK = N // P  # 64 j-blocks
ICH = NL // P  # 8 local i-chunks
G = 16         # j-blocks per superblock
NSB = JBLK // G
NT = N // 512  # 16 column tiles for phase 1
DA = D + 2     # augmented width: [h | 1 | 0pad] (f32r matmul needs even moving dim)
EXP_SHIFT = -20.0

F32 = mybir.dt.float32
F32R = mybir.dt.float32r
BF16 = mybir.dt.bfloat16
F8 = mybir.dt.float8e4
I16 = mybir.dt.int16
AF = mybir.ActivationFunctionType
ALU = mybir.AluOpType
DR = mybir.MatmulPerfMode.DoubleRow

_BUILD_CACHE = {}

LAST_RESULT = None      # BassKernelResults of the most recent run (for test.py)


def _build(S, gb):
    """Build the per-core Bass program. S = adjacency slot count, gb = gate bias."""
    nc = bacc.Bacc(None, target_bir_lowering=False)

    xT_d = nc.dram_tensor("xT", (D, N), BF16, kind="ExternalInput")
    xtl_d = nc.dram_tensor("xtl", (D, NL), BF16, kind="ExternalInput")
    xloc_d = nc.dram_tensor("xloc", (NL, D), F32, kind="ExternalInput")
    wt_d = nc.dram_tensor("wt", (D, DA), BF16, kind="ExternalInput")
    bcol_d = nc.dram_tensor("bcol", (D, 1), F32, kind="ExternalInput")
    wgt_d = nc.dram_tensor("wgt", (D, D), BF16, kind="ExternalInput")
    bg_d = nc.dram_tensor("bg", (D, 1), F32, kind="ExternalInput")
    gwx_d = nc.dram_tensor("gwx", (1, D), F32, kind="ExternalInput")
    gwh_d = nc.dram_tensor("gwh", (1, D), F32, kind="ExternalInput")
    adj_d = nc.dram_tensor("adj", (P, JBLK * S), I16, kind="ExternalInput")
    out_d = nc.dram_tensor("out", (NL, D), F32, kind="ExternalOutput")

    with tile.TileContext(nc) as tc:
        with (
            tc.tile_pool(name="const", bufs=1) as cp,
            tc.tile_pool(name="maskp", bufs=5) as maskp,
            tc.tile_pool(name="work", bufs=3) as wp,
            tc.tile_pool(name="small", bufs=4) as smallp,
        ):
            nc.gpsimd.load_library(library_config.local_scatter)

            # ---- persistent constants / tensors ----
            wt = [cp.tile([P, DA], BF16, tag=f"wt{k}", name=f"wt{k}") for k in range(2)]
            bcol = [cp.tile([P, 1], F32, tag=f"bcol{k}", name=f"bcol{k}") for k in range(2)]
            wgt = [cp.tile([P, D], BF16, tag=f"wgt{k}", name=f"wgt{k}") for k in range(2)]
            bg = [cp.tile([P, 1], F32, tag=f"bg{k}", name=f"bg{k}") for k in range(2)]
            ones_bf = cp.tile([P, S], BF16, tag="ones_bf")
            gwx_b = cp.tile([P, D], F32, tag="gwx_b")
            gwh_b = cp.tile([P, D], F32, tag="gwh_b")
            adj_sb = cp.tile([P, JBLK * S], I16, tag="adj_sb")
            hT8 = cp.tile([P, 2, N], F8, tag="hT8")
            hT_bf = cp.tile([P, 2, N], BF16, tag="hT_bf")
            gt8h = cp.tile([P, 2, NL], F8, tag="gt8h")
            gt8l = cp.tile([P, 2, NL], F8, tag="gt8l")
            haug0 = cp.tile([P, JBLK, P], BF16, tag="haug0")
            haug1 = cp.tile([P, JBLK, P], BF16, tag="haug1")
            ones2 = cp.tile([P, 2], BF16, tag="ones2")
            hacc = [cp.tile([P, DA], F32, tag=f"hacc{i}", name=f"hacc{i}")
                    for i in range(ICH)]
            expsh = cp.tile([P, 1], F32, tag="expsh")
            gbt = cp.tile([P, 1], F32, tag="gbt")
            xls = [cp.tile([P, D], F32, tag=f"xl{i}", name=f"xl{i}")
                   for i in range(ICH)]
            sxs = [cp.tile([P, 1], F32, tag=f"sx{i}", name=f"sx{i}")
                   for i in range(ICH)]
            sxg = [cp.tile([P, 1], F32, tag=f"sxg{i}", name=f"sxg{i}")
                   for i in range(ICH)]

            for k in range(2):
                nc.sync.dma_start(wt[k][:], wt_d[P * k:P * k + P, :])
            nc.vector.memset(ones_bf[:], 1.0)
            tblw = cp.tile([P, 1], F32, tag="tblw")
            nc.vector.memset(tblw[:], 0.0)
            nc.scalar.activation(tblw[:], tblw[:], AF.Identity, bias=0.0, scale=1.0)
            nc.scalar.activation(tblw[:], tblw[:], AF.Exp, bias=0.0, scale=1.0)
            nc.vector.memset(ones2[:, 0:1], 1.0)
            nc.vector.memset(ones2[:, 1:2], 0.0)
            nc.vector.memset(expsh[:], EXP_SHIFT)
            nc.vector.memset(gbt[:], float(gb))

            # ---- merged software pipeline -------------------------------
            # Phase-1 h/h_aug tile-work, the masked-exp e^T stream, and the
            # aggregation chains are interleaved in one PE-paced pipeline:
            #   unit k (k = 0..63 j-blocks):
            #     every 4th unit: phase-1 work for x-tile nt = k/4 + 2
            #     e-unit k: mask scatter, fp8 DoubleRow e^T, exp, mask-mult
            #     odd units: one aggregation chain of superblock k/16 - 1
            # e^T for j-block k only needs hT8 columns from x-tile k/4 (done
            # 8 units earlier), so phase 1 streams through the pipeline.
            with tc.tile_pool(name="xtp", bufs=3) as xtp, \
                 tc.tile_pool(name="ptp", bufs=1) as ptp, \
                 tc.tile_pool(name="pmm", bufs=3, space="PSUM") as pmm, \
                 tc.tile_pool(name="pagg", bufs=2, space="PSUM") as pagg:
                xtl = [cp.tile([P, NL], BF16, tag=f"xtl{k}", name=f"xtl{k}")
                       for k in range(2)]
                pts = [[ptp.tile([P, NL], BF16, tag=f"pt{b}_{g}", name=f"pt{b}_{g}")
                        for g in range(G)] for b in range(2)]

                # x^T tiles come in one fused DMA per nt ([P, 2, 512]; plane =
                # 128-row block of x^T) to halve the SP DMA-issue load
                def xt_dma(nt, eng=None):
                    xt = xtp.tile([P, 2, 512], BF16, tag="xt", name="xt")
                    (eng or nc.sync).dma_start(
                        xt[:],
                        xT_d[:, 512 * nt:512 * nt + 512].rearrange(
                            "(t p) n -> p t n", t=2
                        ),
                    )
                    return xt

                xts0 = xt_dma(0)
                # remaining inputs issue from the (idle) gpsimd queue so they
                # never delay the SP xt stream
                for k in range(2):
                    nc.gpsimd.dma_start(xtl[k][:], xtl_d[P * k:P * k + P, :])
                    nc.gpsimd.dma_start(wgt[k][:], wgt_d[P * k:P * k + P, :])
                    nc.gpsimd.dma_start(bg[k][:], bg_d[P * k:P * k + P, :])
                nc.gpsimd.dma_start(adj_sb[:], adj_d[:])
                nc.gpsimd.dma_start(gwx_b[:], gwx_d[:].to_broadcast((P, D)))
                nc.gpsimd.dma_start(gwh_b[:], gwh_d[:].to_broadcast((P, D)))

                def ph1_unit(nt, xts):
                    # h^T tile in bf16 (transpose source) + fp8 cast for e^T.
                    # All matmuls issue before the ACT reads: the shared psum
                    # tile is tracked whole, so an early ACT read would stall
                    # the second dc's matmuls behind it.
                    ps = pmm.tile([P, NL], F32, tag="mm", name="ps1")
                    for dc in range(2):
                        sl = ps[:, 512 * dc:512 * dc + 512]
                        for k in range(2):
                            nc.tensor.matmul(
                                sl, wt[k][:, P * dc:P * dc + P], xts[:, k, :],
                                start=(k == 0), stop=(k == 1),
                            )
                    nc.scalar.activation(
                        hT_bf[:, 0, 512 * nt:512 * nt + 512],
                        ps[:, 0:512],
                        AF.Identity, bias=bcol[0][:], scale=1.0,
                    )
                    nc.vector.tensor_scalar_add(
                        hT_bf[:, 1, 512 * nt:512 * nt + 512],
                        ps[:, 512:1024], bcol[1][:],
                    )
                    for dc in range(2):
                        nc.gpsimd.tensor_copy(
                            hT8[:, dc, 512 * nt:512 * nt + 512],
                            hT_bf[:, dc, 512 * nt:512 * nt + 512],
                        )

                def ph1_tr(nt):
                    # h_aug rows via xbar DMA transpose of hT_bf (one call per
                    # dc covers 4 j-blocks). The per-dc halves live in separate
                    # fully-contiguous tensors: the xbar path mis-writes
                    # strided outputs on hardware.
                    for dc, dst in ((0, haug0), (1, haug1)):
                        nc.sync.dma_start_transpose(
                            dst[:, 4 * nt:4 * nt + 4, :],
                            hT_bf[:, dc, 512 * nt:512 * nt + 512],
                        )

                def ph2():
                    # g^T = (A_sym^T W) x_loc^T + A_sym^T b (host-folded), as an
                    # fp8 (hi, lo) pair: hi = fp8(g), lo = fp8(g - hi)
                    for dc in range(2):
                        for ih in range(2):
                            ps = pmm.tile([P, NL], F32, tag="mm", name="ps2g")
                            sl = ps[:, 0:512]
                            for k in range(2):
                                nc.tensor.matmul(
                                    sl, wgt[k][:, P * dc:P * dc + P],
                                    xtl[k][:, 512 * ih:512 * ih + 512],
                                    start=(k == 0), stop=(k == 1),
                                )
                            hi = gt8h[:, dc, 512 * ih:512 * ih + 512]
                            nc.scalar.activation(
                                hi, sl, AF.Identity, bias=bg[dc][:], scale=1.0,
                            )
                            nc.vector.scalar_tensor_tensor(
                                out=gt8l[:, dc, 512 * ih:512 * ih + 512],
                                in0=sl, scalar=bg[dc][:], in1=hi,
                                op0=ALU.add, op1=ALU.subtract,
                            )

                def e_unit(k):
                    sb, g = k // G, k % G
                    mask_t = maskp.tile([P, NL], BF16, tag="mask", name="mask_t")
                    nc.gpsimd.local_scatter(
                        mask_t[:], ones_bf[:], adj_sb[:, S * k:S * k + S],
                        channels=P, num_elems=NL, num_idxs=S,
                    )
                    ps = pmm.tile([P, NL], F32, tag="mm", name="ps3")
                    for ih in range(2):
                        for gsrc, st_, sp_ in ((gt8h, True, False), (gt8l, False, True)):
                            nc.tensor.matmul(
                                ps[:, 512 * ih:512 * ih + 512],
                                hT8[:, :, P * k:P * k + P],
                                gsrc[:, :, 512 * ih:512 * ih + 512],
                                start=st_, stop=sp_, perf_mode=DR,
                            )
                    pt = pts[sb % 2][g]
                    nc.scalar.activation(pt[:], ps[:], AF.Exp, bias=expsh[:], scale=1.0)
                    nc.vector.tensor_tensor(pt[:], pt[:], mask_t[:], op=ALU.mult)

                def agg_seg(sbi, ic, g0, glen):
                    # a segment of a j-superblock's aggregation chain; early
                    # segments run inside their own superblock (quarters for
                    # sb0 to fill the pipeline warmup), later ones in the next
                    # superblock -- keeps the PE fed early and shortens the tail
                    buf = sbi % 2
                    psa = pagg.tile([P, DA], F32, tag="agg", name="psa")
                    for t in range(glen):
                        g = g0 + t
                        pt = pts[buf][g][:, P * ic:P * ic + P]
                        jb = sbi * G + g
                        nc.tensor.matmul(psa[:, 0:P], pt, haug0[:, jb, :],
                                         start=(t == 0), stop=False)
                        nc.tensor.matmul(psa[:, P:D], pt, haug1[:, jb, :],
                                         start=False, stop=False)
                        nc.tensor.matmul(psa[:, D:DA], pt, ones2[:],
                                         start=False, stop=(t == glen - 1))
                    if sbi == 0 and g0 == 0:
                        nc.vector.tensor_copy(hacc[ic][:], psa[:])
                    else:
                        nc.vector.tensor_tensor(
                            hacc[ic][:], psa[:], hacc[ic][:], op=ALU.add
                        )

                def agg_half(sbi, ic, half):
                    agg_seg(sbi, ic, half * (G // 2), G // 2)

                def ph4(ic):
                    zrec = smallp.tile([P, 1], F32, tag="zrec", name="zrec")
                    nc.vector.reciprocal(zrec[:], hacc[ic][:, D:D + 1])
                    hp = wp.tile([P, D], F32, tag="hp", name="hp")
                    nc.scalar.activation(hp[:], hacc[ic][:, 0:D], AF.Relu,
                                         bias=0.0, scale=zrec[:])
                    scr2 = wp.tile([P, D], F32, tag="scr", name="scr2")
                    sh = smallp.tile([P, 1], F32, tag="sh", name="sh")
                    nc.gpsimd.tensor_tensor(scr2[:], hp[:], gwh_b[:], op=ALU.mult)
                    nc.vector.reduce_sum(sh[:], scr2[:], axis=mybir.AxisListType.X)
                    cf = smallp.tile([P, 1], F32, tag="cf", name="cf")
                    nc.scalar.activation(cf[:], sh[:], AF.Sigmoid, bias=sxg[ic][:], scale=1.0)
                    dif = wp.tile([P, D], F32, tag="scr", name="dif")
                    nc.gpsimd.tensor_tensor(dif[:], xls[ic][:], hp[:], op=ALU.subtract)
                    ot = wp.tile([P, D], F32, tag="ot", name="ot")
                    nc.vector.scalar_tensor_tensor(
                        out=ot[:], in0=dif[:], scalar=cf[:], in1=hp[:],
                        op0=ALU.mult, op1=ALU.add,
                    )
                    nc.sync.dma_start(out_d[P * ic:P * ic + P, :], ot[:])

                # -- schedule --
                xts1 = xt_dma(1, nc.scalar)
                for k in range(2):
                    nc.sync.dma_start(bcol[k][:], bcol_d[P * k:P * k + P, :])
                ph1_unit(0, xts0)
                ph2()
                ph1_unit(1, xts1)
                nt_next = 2
                pending_xt = xt_dma(nt_next) if nt_next < NT else None
                for k in range(JBLK):
                    if k % 4 == 2 and 10 <= k <= 38:
                        # hoisted x-half of the gate (only needed by phase 4);
                        # one i-chunk per phase-1 slot keeps every queue clear
                        ic = (k - 10) // 4
                        nc.sync.dma_start(xls[ic][:],
                                          xloc_d[P * ic:P * ic + P, :])
                        scr = wp.tile([P, D], F32, tag="scr", name="scrx")
                        nc.vector.tensor_tensor(scr[:], xls[ic][:],
                                                gwx_b[:], op=ALU.mult)
                        nc.vector.reduce_sum(sxs[ic][:], scr[:],
                                             axis=mybir.AxisListType.X)
                        nc.vector.tensor_tensor(sxg[ic][:], sxs[ic][:],
                                                gbt[:], op=ALU.add)
                    if k % 4 == 0:
                        if nt_next < NT:
                            xts = pending_xt
                            nt = nt_next
                            nt_next += 1
                            pending_xt = xt_dma(nt_next) if nt_next < NT else None
                            ph1_unit(nt, xts)
                        if k // 4 - 1 + 2 <= NT:
                            tr_nt = k // 4
                            if tr_nt < NT:
                                ph1_tr(tr_nt)
                    e_unit(k)
                    sb, g = k // G, k % G
                    if sb == 0:
                        # quarter-chains as soon as their pts exist
                        if 6 <= g <= 13:
                            agg_seg(0, g - 6, 0, 4)
                        if 10 <= g <= 15:
                            agg_seg(0, g - 10, 4, 4)
                    elif sb == 1:
                        if g == 0:
                            agg_seg(0, 6, 4, 4)
                        elif g == 1:
                            agg_seg(0, 7, 4, 4)
                        elif 2 <= g <= 9:
                            agg_half(0, g - 2, 1)
                        if g >= 9:
                            agg_half(1, g - 9, 0)
                    else:
                        if g == 0:
                            agg_half(sb - 1, 7, 0)
                        elif 1 <= g <= 8:
                            agg_half(sb - 1, g - 1, 1)
                        if g >= 9:
                            agg_half(sb, g - 9, 0)
                # tail: last first-half chain, then second halves + phase 4
                agg_half(NSB - 1, 7, 0)
                for ic in range(ICH):
                    agg_half(NSB - 1, ic, 1)
                    ph4(ic)

    nc.compile()
    return nc
def _prep_adjacency(edge_index):
    """Compact per-(core, dst) slot lists of local source columns, deduped,
    with self loops. Returns (adj_idx [M, N, S] int16, S)."""
    s_arr = np.asarray(edge_index[0], dtype=np.int64)
    d_arr = np.asarray(edge_index[1], dtype=np.int64)
    keys = s_arr * N + d_arr
    loops = np.arange(N, dtype=np.int64) * (N + 1)
    allk = np.unique(np.concatenate([keys, loops]))
    src = allk // N
    dst = allk % N
    core = src // NL
    sloc = (src % NL).astype(np.int16)
    group = core * N + dst
    order = np.argsort(group, kind="stable")
    gs = group[order]
    sl = sloc[order]
    _, first, counts = np.unique(gs, return_index=True, return_counts=True)
    smax = int(counts.max())
    S = max(smax + (smax % 2), 2)
    ranks = np.arange(len(gs)) - np.repeat(first, counts)
    adj_idx = np.full((M, N, S), -1, np.int16)
    adj_idx[gs // N, gs % N, ranks] = sl
    return np.ascontiguousarray(adj_idx), S


def prepare(x, edge_index, W_w, W_b, A, gate_w, gate_b):
    """Host prep: build (or fetch cached) program + per-core input maps."""
    x = np.ascontiguousarray(np.asarray(x, dtype=np.float32))
    W_w = np.asarray(W_w, dtype=np.float32)
    W_b = np.asarray(W_b, dtype=np.float32)
    A = np.asarray(A, dtype=np.float32)
    gate_w = np.asarray(gate_w, dtype=np.float32)
    gb = float(np.asarray(gate_b).reshape(-1)[0])
    assert x.shape == (N, D) and edge_index.shape[1] >= 1

    adj_idx, S = _prep_adjacency(edge_index)

    key = (S, gb)
    if key not in _BUILD_CACHE:
        _BUILD_CACHE[key] = _build(S, gb)
    nc = _BUILD_CACHE[key]

    import ml_dtypes
    xT = np.ascontiguousarray(x.T.astype(ml_dtypes.bfloat16))
    wt_aug = np.ascontiguousarray(
        np.concatenate([W_w.T, np.zeros((D, 2), np.float32)], axis=1)
    ).astype(ml_dtypes.bfloat16)
    bcol = np.ascontiguousarray(W_b.reshape(D, 1))
    asym = (A + A.T).astype(np.float32)
    wgt = np.ascontiguousarray(W_w.T @ asym).astype(ml_dtypes.bfloat16)
    bg = np.ascontiguousarray((asym.T @ W_b).reshape(D, 1))
    gwx = np.ascontiguousarray(gate_w[:, :D])
    gwh = np.ascontiguousarray(gate_w[:, D:])

    in_maps = []
    for c in range(M):
        xl = x[c * NL:(c + 1) * NL]
        in_maps.append(dict(
            xT=xT,
            xtl=np.ascontiguousarray(xl.T.astype(ml_dtypes.bfloat16)),
            xloc=np.ascontiguousarray(xl),
            wt=wt_aug, bcol=bcol, wgt=wgt, bg=bg,
            gwx=gwx, gwh=gwh,
            adj=np.ascontiguousarray(
                adj_idx[c].reshape(JBLK, P, S).transpose(1, 0, 2).reshape(P, JBLK * S)
            ),
        ))
    return nc, in_maps


def kernel(x, edge_index, W_w, W_b, A, gate_w, gate_b):
    global LAST_RESULT
    nc, in_maps = prepare(x, edge_index, W_w, W_b, A, gate_w, gate_b)
    # this axon client has no NTFF hook; a BASS_TRACE env would crash the run
    os.environ["BASS_NEVER_TRACE"] = "1"
    res = run_bass_kernel_spmd(nc, in_maps, core_ids=list(range(M)))
    LAST_RESULT = res
    out = np.concatenate([res.results[c]["out"] for c in range(M)], axis=0)
    return out

